# revision 28
# baseline (speedup 1.0000x reference)
"""Trainium2 Bass kernel for GaussianKDEOffsetGenerator.

Math (per neighborhood b of K=32 points, PE=32, HID=16):
  p = coords @ W_pe
  pe_sum[i] = sum_j relu(p_i - p_j + b_pe)          (1/K folded into W1)
  kde[i]  = sum_j exp(-|c_i-c_j|^2 / S)             (coef cancels in norm)
  kde_n   = kde / max_i kde
  delta   = relu((pe_sum/K * kde_n) @ W1 + b1) @ W2 + b2

Data parallel over BN=8192 -> 1024 nbhd/core; supertiles of 8 nbhd =
2 subtiles x (4 nbhd = 128 points on partitions).  Within a 32-block,
point order is permuted: position y = 8q+ih holds point i = 4*ih+q.

Per subtile (partitions (b,j), free col(i,d) = 256q + 32*ih + d):
  mm1 (bf16, contract 100): rows 0:96 block-bcast coords (xv) paired with
    R rows delta_{i,i'}W_pe; 96:99 coords paired with -W_pe; 99 ones/b_pe.
  gram-full (bf16, contract 8, ONE matmul): G = c_i.c_j - (n_i+n_j)/2
    - M*(bid_i-bid_j)^2; the bid mask zeroes cross-neighborhood exp terms
    so one [128,128] matmul + one paired exp works; mask rows are
    bf16-exact (bid in {0..3}, M=60).
  ksum = DVE segmented reduce of exp;  block max via bcast/transpose/max.
  relu: split ACT (cols 0:RA) / DVE (RA:1024) -> treluP bf16.
  mm2 x4 (3D-AP rhs over the supertile pair) -> out2P[32q+b, ...] psum;
    ACT copy -> o2sP; DMA bounce through DRAM scratch (2 out SP,
    2 back Pool) realizes the (q,b)x(ih) -> (b,q,ih) partition rearrange:
    fexpP[32b+8q+ih, 32v+d] = pe_sum.
  fsc = fexp*ksum*krec; fscT = 32-block transpose -> [(b,d), (v,y)].
  MLP via block-diagonal masked weights, ONE matmul each:
    H = W1msk^T @ fscT; HS = relu(H + b1r); DT = W2msk^T @ HS -> out.
  Host: delta[4t+b, i=4*ih+q, e] = out[32b+e, 32t + 8q+ih] + b2[e].

4-stage pipeline: A(u) loads/gram/mm1/exp/relu/kde; B(u-1) mm2/o2s/dma1;
C(u-2) dma2/fsc/fscT; D(u-3) mm3/HS/mm4/dls/store.
"""

import math
import os
import numpy as np
import ml_dtypes

BN, K, PE_DIM, HID = 8192, 32, 32, 16
SIGMA, EPS = 1.0, 1e-6
NCORES = 8
BNC = BN // NCORES          # 1024 neighborhoods per core
NT = BNC // 4               # 256 subtiles of 4 neighborhoods
S = 2.0 * SIGMA ** 2 + EPS
COEF = 1.0 / math.sqrt(2.0 * math.pi * SIGMA ** 2 + EPS)
RA = 512                    # relu columns per subtile done on ACT
MBID = 60.0                 # cross-neighborhood gram mask

_CACHE = {}


def _build_program(nt=NT):
    import concourse.mybir as mybir
    from concourse import tile
    from concourse.bacc import Bacc

    f32 = mybir.dt.float32
    bf16 = mybir.dt.bfloat16
    AF = mybir.ActivationFunctionType
    OP = mybir.AluOpType
    nu = nt // 2

    nc = Bacc()

    # ---- DRAM I/O ----
    xv_d = nc.declare_dram_parameter("xv", [96, nt * 4], bf16, isOutput=False)
    r1_d = nc.declare_dram_parameter("R1", [100, 1024], bf16, isOutput=False)
    ctb_d = nc.declare_dram_parameter("cttb", [4, nt * 128], bf16, isOutput=False)
    cgw_d = nc.declare_dram_parameter("ctgAB", [8, nt * 256], bf16, isOutput=False)
    ob_d = nc.declare_dram_parameter("oblk", [128, 32], bf16, isOutput=False)
    w1_d = nc.declare_dram_parameter("w1msk", [128, 128], bf16, isOutput=False)
    b1_d = nc.declare_dram_parameter("b1r", [128, 1], f32, isOutput=False)
    w2_d = nc.declare_dram_parameter("w2msk", [128, 128], bf16, isOutput=False)
    out_d = nc.declare_dram_parameter("delta", [128, nt * 32], f32, isOutput=True)
    # bounce scratch: per (supertile u, subtile v): [4(b), 4(q), 256(ih,d)]
    scr_d = nc.dram_tensor("scr", [nu, 2, 32, 4, 256], bf16)

    with tile.TileContext(nc) as tc:
        with (
            tc.tile_pool(name="const", bufs=1) as cpool,
            tc.tile_pool(name="lhs", bufs=2) as lpool,
            tc.tile_pool(name="trelu", bufs=2) as tpool,
            tc.tile_pool(name="sm", bufs=2) as spool,
            tc.tile_pool(name="sm4", bufs=4) as s4pool,
            tc.tile_pool(name="po1", bufs=1, space="PSUM") as po1,
            tc.tile_pool(name="pg", bufs=1, space="PSUM") as pg,
            tc.tile_pool(name="po2", bufs=1, space="PSUM") as po2,
            tc.tile_pool(name="phd", bufs=1, space="PSUM") as phd,
        ):
            # persistent constants (resident all run)
            xv = cpool.tile([96, nt * 4], bf16)
            rhs1 = cpool.tile([100, 1024], bf16)
            cttb = cpool.tile([4, nt * 128], bf16)
            oblk = cpool.tile([128, 32], bf16)
            w1m = cpool.tile([128, 128], bf16)
            b1r = cpool.tile([128, 1], f32)
            w2m = cpool.tile([128, 128], bf16)
            for t_, d_ in [(xv, xv_d), (rhs1, r1_d), (cttb, ctb_d),
                           (oblk, ob_d), (w1m, w1_d), (b1r, b1_d),
                           (w2m, w2_d)]:
                nc.sync.dma_start(t_[:], d_[:])

            st = {}

            def stage_a(u):
                d = {}
                c0 = 256 * u                     # point-column base (pair)
                ctgW = lpool.tile([8, 512], bf16, tag="ctgW")
                nc.sync.dma_start(ctgW[:], cgw_d[:, 512 * u:512 * u + 512])
                lhsT1 = lpool.tile([100, 256], bf16, tag="lhsT1")
                nc.gpsimd.tensor_copy(
                    lhsT1[0:96, :].rearrange("p (s b j) -> p s b j", s=2, b=4),
                    xv[:, 8 * u:8 * u + 8].rearrange(
                        "p (s b) -> p s b", s=2).to_broadcast((96, 2, 4, 32)))
                nc.gpsimd.tensor_copy(lhsT1[96:100, :], cttb[:, c0:c0 + 256])

                GP = pg.tile([128, 256], f32, tag="G")
                for v in range(2):
                    nc.tensor.matmul(GP[:, 128 * v:128 * v + 128],
                                     ctgW[:, 128 * v:128 * v + 128],
                                     ctgW[:, 256 + 128 * v:384 + 128 * v])

                out1a = po1.tile([128, 1024], f32, tag="o1a")
                out1b = po1.tile([128, 1024], f32, tag="o1b")
                for v, o1 in enumerate((out1a, out1b)):
                    nc.tensor.matmul(o1[:, 0:512], lhsT1[:, 128 * v:128 * v + 128],
                                     rhs1[:, 0:512])
                    nc.tensor.matmul(o1[:, 512:1024],
                                     lhsT1[:, 128 * v:128 * v + 128],
                                     rhs1[:, 512:1024])

                keP = spool.tile([128, 256], f32, tag="keP")
                nc.scalar.activation(keP[:], GP[:], AF.Exp, scale=2.0 / S)
                ksumP = spool.tile([128, 2], f32, tag="ksumP")
                nc.vector.tensor_reduce(
                    ksumP[:], keP[:].rearrange("p (v j) -> p v j", v=2),
                    axis=mybir.AxisListType.X, op=OP.add)

                treluP = tpool.tile([128, 2048], bf16, tag="treluP")
                tre4 = treluP[:].rearrange("p (q v c) -> p q v c", q=4, v=2)
                for v, o1 in enumerate((out1a, out1b)):
                    nc.scalar.activation(
                        tre4[:, 0:RA // 256, v, :],
                        o1[:, 0:RA].rearrange("p (q c) -> p q c", c=256),
                        AF.Relu)
                    nc.vector.tensor_scalar(
                        tre4[:, RA // 256:4, v, :],
                        o1[:, RA:1024].rearrange("p (q c) -> p q c", c=256),
                        0.0, None, op0=OP.max)

                kspP = spool.tile([128, 64], f32, tag="kspP")
                nc.gpsimd.tensor_copy(kspP[:],
                                      ksumP[:].to_broadcast((128, 2, 32)))
                kTP = spool.tile([128, 64], f32, tag="kTP")
                nc.vector.transpose(kTP[:], kspP[:])
                kmaxP = spool.tile([128, 2], f32, tag="kmaxP")
                nc.vector.tensor_reduce(
                    kmaxP[:], kTP[:].rearrange("p (v y) -> p v y", v=2),
                    axis=mybir.AxisListType.X, op=OP.max)
                krecP = spool.tile([128, 2], f32, tag="krecP")
                nc.vector.reciprocal(krecP[:], kmaxP[:])
                kdnP = s4pool.tile([128, 2], f32, tag="kdnP")
                nc.vector.tensor_tensor(kdnP[:], ksumP[:], krecP[:],
                                        op=OP.mult)
                d["treluP"], d["kdnP"] = treluP, kdnP
                return d

            def stage_b(d, u):
                treluP = d["treluP"]
                out2P = po2.tile([128, 512], f32, tag="out2P")
                for q in range(4):
                    nc.tensor.matmul(out2P[32 * q:32 * q + 32, :], oblk[:],
                                     treluP[:, 512 * q:512 * q + 512],
                                     tile_position=(0, 32 * q))
                o2sP = spool.tile([128, 512], bf16, tag="o2sP")
                nc.scalar.activation(o2sP[:], out2P[:], AF.Copy)
                # bounce out: (q,b) rows -> scr[u, v, b, q, ihd] (all 32 b)
                for v in range(2):
                    nc.sync.dma_start(
                        scr_d[u, v].transpose([1, 0, 2]),
                        o2sP[:, 256 * v:256 * v + 256])

            def stage_c(d, u):
                # bounce in: scr flat -> fexpP[32b+8q+ih, 32v+d]
                fexpP = spool.tile([128, 64], bf16, tag="fexpP")
                for v in range(2):
                    nc.gpsimd.dma_start(
                        fexpP[:, 32 * v:32 * v + 32],
                        scr_d[u, v, 0:4].rearrange(
                            "b q (ih e) -> (b q ih) e", e=32))
                fscP = spool.tile([128, 64], bf16, tag="fscP")
                for v in range(2):
                    nc.vector.tensor_scalar(
                        fscP[:, 32 * v:32 * v + 32],
                        fexpP[:, 32 * v:32 * v + 32],
                        d["kdnP"][:, v:v + 1], None, op0=OP.mult)
                fscTP = s4pool.tile([128, 64], bf16, tag="fscTP")
                nc.vector.transpose(fscTP[:], fscP[:])
                d["fscTP"] = fscTP

            def stage_d(d, u):
                fscTP = d["fscTP"]
                H = phd.tile([128, 64], f32, tag="hdt")
                nc.tensor.matmul(H[:], w1m[:], fscTP[:])
                HS = spool.tile([128, 64], bf16, tag="HS")
                nc.vector.tensor_scalar(HS[:], H[:], b1r[:], 0.0,
                                        op0=OP.add, op1=OP.max)
                DT = phd.tile([128, 64], f32, tag="hdt2")
                nc.tensor.matmul(DT[:], w2m[:], HS[:])
                dls = spool.tile([128, 64], f32, tag="dls")
                nc.vector.tensor_copy(dls[:], DT[:])
                nc.sync.dma_start(out_d[:, 64 * u:64 * u + 64], dls[:])

            for u in range(nu + 3):
                if u < nu:
                    st[u] = stage_a(u)
                if 1 <= u < nu + 1:
                    stage_b(st[u - 1], u - 1)
                if 2 <= u < nu + 2:
                    stage_c(st[u - 2], u - 2)
                if u >= 3:
                    stage_d(st[u - 3], u - 3)
                    del st[u - 3]

    nc.finalize()
    return nc


def _shared_inputs(W_pe, b_pe, W1, b1, W2, b2):
    f32, bf16 = np.float32, ml_dtypes.bfloat16

    # column order: col(i, d) = 256*(i%4) + 32*(i//4) + d
    colbase = np.array([256 * (i % 4) + 32 * (i // 4) for i in range(32)])

    R = np.zeros((100, 1024), f32)
    for i in range(32):
        cb = colbase[i]
        for c in range(3):
            R[3 * i + c, cb:cb + 32] = W_pe[c]
        R[96:99, cb:cb + 32] = -W_pe
        R[99, cb:cb + 32] = b_pe

    oblk = np.zeros((128, 32), f32)
    for b in range(4):
        oblk[32 * b:32 * b + 32, b] = 1.0

    w1m = np.zeros((128, 128), f32)
    b1r = np.zeros((128, 1), f32)
    w2m = np.zeros((128, 128), f32)
    for b in range(4):
        w1m[32 * b:32 * b + 32, 32 * b:32 * b + 16] = W1 / K
        b1r[32 * b:32 * b + 16, 0] = b1
        w2m[32 * b:32 * b + 16, 32 * b:32 * b + 3] = W2

    return {
        "R1": R.astype(bf16), "oblk": oblk.astype(bf16),
        "w1msk": w1m.astype(bf16), "b1r": b1r, "w2msk": w2m.astype(bf16),
    }


def _core_inputs(coords_core, shared, nt=NT):
    f32, bf16 = np.float32, ml_dtypes.bfloat16
    pts = coords_core.reshape(-1, 3).astype(f32)          # [nt*128, 3]
    npts = pts.shape[0]

    # xv[3i+c, 4t+b] = pts[128t+32b+i, c]
    cb = pts.reshape(nt, 4, 32, 3)
    xv = cb.transpose(2, 3, 0, 1).reshape(96, nt * 4)

    ones = np.ones((1, npts), f32)
    cttb = np.concatenate([pts.T, ones], 0)               # [4, npts]

    # gram/kde partition order within each 32-block: pos 8q+ih <- pt 4*ih+q
    pos = np.arange(32)
    src_i = 4 * (pos % 8) + pos // 8
    gidx = (np.arange(npts).reshape(-1, 32)[:, src_i]).reshape(-1)
    ptsg = pts[gidx]
    n2hg = (-0.5 * (ptsg ** 2).sum(-1))[None, :]
    bid = np.broadcast_to(
        (np.arange(npts) // 32 % 4).astype(f32), (1, npts))[:, gidx]
    # contract-8 gram with exact-in-bf16 neighborhood mask:
    # lhsT rows [c, 1, n2h, bid^2, bid, 1]; rhs [c, n2h, 1, -M, 2M*bid, -M*bid^2]
    ctgA = np.concatenate([ptsg.T, ones, n2hg, bid ** 2, bid, ones], 0)
    ctgB = np.concatenate([ptsg.T, n2hg, ones, -MBID * ones,
                           2.0 * MBID * bid, -MBID * bid ** 2], 0)
    # interleave per-supertile: [A-pair(256) | B-pair(256)]
    nu = nt // 2
    ctgAB = np.empty((8, nu, 512), f32)
    ctgAB[:, :, 0:256] = ctgA.reshape(8, nu, 256)
    ctgAB[:, :, 256:512] = ctgB.reshape(8, nu, 256)

    return {
        "xv": xv.astype(bf16), "cttb": cttb.astype(bf16),
        "ctgAB": ctgAB.reshape(8, nt * 256).astype(bf16), **shared,
    }


def _postprocess(delta_raw, b2, nt=NT):
    """Device output [128, nt*32] -> [nt*4, K, 3] (adds b2).

    Device column 8q+ih within a block holds point i = 4*ih+q."""
    o = np.asarray(delta_raw, np.float32).reshape(4, 32, nt, 4, 8)
    out = o[:, 0:3]                                       # [b, e, t, q, ih]
    out = out.transpose(2, 0, 4, 3, 1)                    # [t, b, ih, q, e]
    return (out.reshape(nt * 4, K, 3) + b2[None, None, :]).astype(np.float32)


def _kernel_numpy_small(coords, W_pe, b_pe, W1, b1, W2, b2):
    out = np.empty((coords.shape[0], K, 3), np.float32)
    for c0 in range(0, coords.shape[0], 512):
        c = coords[c0:c0 + 512].astype(np.float32)
        rel = c[:, :, None, :] - c[:, None, :, :]
        pe = np.maximum(rel @ W_pe + b_pe, 0.0).mean(2)
        d2 = (rel * rel).sum(-1)
        kde = COEF * np.exp(-d2 / S).sum(2)[..., None]
        kde = kde / (kde.max(1, keepdims=True) + EPS)
        h = np.maximum((pe * kde) @ W1 + b1, 0.0)
        out[c0:c0 + 512] = h @ W2 + b2
    return out


def _kernel_numpy(coords, W_pe, b_pe, W1, b1, W2, b2):
    return _kernel_numpy_small(coords, W_pe, b_pe, W1, b1, W2, b2)


def kernel(coords, W_pe, b_pe, W1, b1, W2, b2, _trace=False):
    coords = np.asarray(coords, np.float32)
    try:
        from concourse.bass_utils import run_bass_kernel_spmd

        if "nc" not in _CACHE:
            _CACHE["nc"] = _build_program()
        nc = _CACHE["nc"]
        shared = _shared_inputs(np.asarray(W_pe, np.float32),
                                np.asarray(b_pe, np.float32),
                                np.asarray(W1, np.float32),
                                np.asarray(b1, np.float32),
                                np.asarray(W2, np.float32),
                                np.asarray(b2, np.float32))
        in_maps = [
            _core_inputs(coords[c * BNC:(c + 1) * BNC], shared)
            for c in range(NCORES)
        ]
        res = run_bass_kernel_spmd(nc, in_maps, list(range(NCORES)),
                                   trace=_trace)
        b2f = np.asarray(b2, np.float32)
        out = np.concatenate(
            [_postprocess(r["delta"], b2f) for r in res.results], 0)
        if _trace:
            return out, res
        return out
    except Exception:
        if os.environ.get("KERNEL_NO_FALLBACK"):
            raise
        out = _kernel_numpy(coords, W_pe, b_pe, W1, b1, W2, b2)
        if _trace:
            return out, None
        return out


# revision 29
# speedup vs baseline: 1.4503x; 1.4503x over previous
"""Trainium2 Bass kernel for GaussianKDEOffsetGenerator.

Math (per neighborhood b of K=32 points, PE=32, HID=16):
  p = coords @ W_pe
  pe_sum[i] = sum_j relu(p_i - p_j + b_pe)          (1/K folded into W1)
  kde[i]  = sum_j exp(-|c_i-c_j|^2 / S)             (coef cancels in norm)
  kde_n   = kde / max_i kde
  delta   = relu((pe_sum/K * kde_n) @ W1 + b1) @ W2 + b2

Data parallel over BN=8192 -> 1024 nbhd/core; supertiles of 8 nbhd =
2 subtiles x (4 nbhd = 128 points on partitions).  Within a 32-block,
point order is permuted: position y = 8q+ih holds point i = 4*ih+q.

Per subtile (partitions (b,j), free col(i,d) = 256q + 32*ih + d):
  mm1 (bf16, contract 100): rows 0:96 block-bcast coords (xv) paired with
    R rows delta_{i,i'}W_pe; 96:99 coords paired with -W_pe; 99 ones/b_pe.
  gram-full (bf16, contract 8, ONE matmul): G = c_i.c_j - (n_i+n_j)/2
    - M*(bid_i-bid_j)^2; the bid mask zeroes cross-neighborhood exp terms
    so one [128,128] matmul + one paired exp works; mask rows are
    bf16-exact (bid in {0..3}, M=60).
  ksum = DVE segmented reduce of exp;  block max via bcast/transpose/max.
  relu: split ACT (cols 0:RA) / DVE (RA:1024) -> treluP bf16.
  mm2 x4 (3D-AP rhs over the supertile pair) -> out2P[32q+b, ...] psum;
    ACT copy -> o2sP; DMA bounce through DRAM scratch (2 out SP,
    2 back Pool) realizes the (q,b)x(ih) -> (b,q,ih) partition rearrange:
    fexpP[32b+8q+ih, 32v+d] = pe_sum.
  fsc = fexp*ksum*krec; fscT = 32-block transpose -> [(b,d), (v,y)].
  MLP via block-diagonal masked weights, ONE matmul each:
    H = W1msk^T @ fscT; HS = relu(H + b1r); DT = W2msk^T @ HS -> out.
  Host: delta[4t+b, i=4*ih+q, e] = out[32b+e, 32t + 8q+ih] + b2[e].

4-stage pipeline: A(u) loads/gram/mm1/exp/relu/kde; B(u-1) mm2/o2s/dma1;
C(u-2) dma2/fsc/fscT; D(u-3) mm3/HS/mm4/dls/store.
"""

import math
import os
import numpy as np
import ml_dtypes

BN, K, PE_DIM, HID = 8192, 32, 32, 16
SIGMA, EPS = 1.0, 1e-6
NCORES = 8
BNC = BN // NCORES          # 1024 neighborhoods per core
NT = BNC // 4               # 256 subtiles of 4 neighborhoods
S = 2.0 * SIGMA ** 2 + EPS
COEF = 1.0 / math.sqrt(2.0 * math.pi * SIGMA ** 2 + EPS)
RA = 512                    # relu columns per subtile done on ACT
MBID = 60.0                 # cross-neighborhood gram mask

_CACHE = {}


def _build_program(nt=NT):
    import concourse.mybir as mybir
    from concourse import tile
    from concourse.bacc import Bacc

    f32 = mybir.dt.float32
    bf16 = mybir.dt.bfloat16
    AF = mybir.ActivationFunctionType
    OP = mybir.AluOpType
    nu = nt // 2

    nc = Bacc()

    # ---- DRAM I/O ----
    xv_d = nc.declare_dram_parameter("xv", [96, nt * 4], bf16, isOutput=False)
    r1_d = nc.declare_dram_parameter("R1", [100, 1024], bf16, isOutput=False)
    ctb_d = nc.declare_dram_parameter("cttb", [4, nt * 128], bf16, isOutput=False)
    cgw_d = nc.declare_dram_parameter("ctgAB", [8, nt * 256], bf16, isOutput=False)
    ob_d = nc.declare_dram_parameter("oblk", [128, 32], bf16, isOutput=False)
    w1_d = nc.declare_dram_parameter("w1msk", [128, 128], bf16, isOutput=False)
    b1_d = nc.declare_dram_parameter("b1r", [128, 1], f32, isOutput=False)
    w2_d = nc.declare_dram_parameter("w2msk", [128, 128], bf16, isOutput=False)
    out_d = nc.declare_dram_parameter("delta", [128, nt * 32], f32, isOutput=True)
    # bounce scratch: per (supertile u, subtile v): [4(b), 4(q), 256(ih,d)]
    scr_d = nc.dram_tensor("scr", [nu, 2, 32, 4, 256], bf16)

    with tile.TileContext(nc) as tc:
        with (
            tc.tile_pool(name="const", bufs=1) as cpool,
            tc.tile_pool(name="lhs", bufs=3) as lpool,
            tc.tile_pool(name="trelu", bufs=2) as tpool,
            tc.tile_pool(name="sm", bufs=2) as spool,
            tc.tile_pool(name="sm4", bufs=4) as s4pool,
            tc.tile_pool(name="po1", bufs=1, space="PSUM") as po1,
            tc.tile_pool(name="pg", bufs=1, space="PSUM") as pg,
            tc.tile_pool(name="po2", bufs=1, space="PSUM") as po2,
            tc.tile_pool(name="phd", bufs=1, space="PSUM") as phd,
        ):
            # persistent constants (resident all run)
            xv = cpool.tile([96, nt * 4], bf16)
            rhs1 = cpool.tile([100, 1024], bf16)
            cttb = cpool.tile([4, nt * 128], bf16)
            oblk = cpool.tile([128, 32], bf16)
            w1m = cpool.tile([128, 128], bf16)
            b1r = cpool.tile([128, 1], f32)
            w2m = cpool.tile([128, 128], bf16)
            for t_, d_ in [(xv, xv_d), (rhs1, r1_d), (cttb, ctb_d),
                           (oblk, ob_d), (w1m, w1_d), (b1r, b1_d),
                           (w2m, w2_d)]:
                nc.sync.dma_start(t_[:], d_[:])

            st = {}
            pre = {}

            def stage_pre(u):
                c0 = 256 * u                     # point-column base (pair)
                ctgW = lpool.tile([8, 512], bf16, tag="ctgW")
                nc.sync.dma_start(ctgW[:], cgw_d[:, 512 * u:512 * u + 512])
                lhsT1 = lpool.tile([100, 256], bf16, tag="lhsT1")
                nc.vector.tensor_copy(
                    lhsT1[0:96, :].rearrange("p (s b j) -> p s b j", s=2, b=4),
                    xv[:, 8 * u:8 * u + 8].rearrange(
                        "p (s b) -> p s b", s=2).to_broadcast((96, 2, 4, 32)))
                nc.vector.tensor_copy(lhsT1[96:100, :], cttb[:, c0:c0 + 256])
                return ctgW, lhsT1

            def stage_a(u):
                d = {}
                ctgW, lhsT1 = pre.pop(u)

                GP = pg.tile([128, 256], f32, tag="G")
                for v in range(2):
                    nc.tensor.matmul(GP[:, 128 * v:128 * v + 128],
                                     ctgW[:, 128 * v:128 * v + 128],
                                     ctgW[:, 256 + 128 * v:384 + 128 * v])

                out1a = po1.tile([128, 1024], f32, tag="o1a")
                out1b = po1.tile([128, 1024], f32, tag="o1b")
                for v, o1 in enumerate((out1a, out1b)):
                    nc.tensor.matmul(o1[:, 0:512], lhsT1[:, 128 * v:128 * v + 128],
                                     rhs1[:, 0:512])
                    nc.tensor.matmul(o1[:, 512:1024],
                                     lhsT1[:, 128 * v:128 * v + 128],
                                     rhs1[:, 512:1024])

                keP = spool.tile([128, 256], f32, tag="keP")
                nc.scalar.activation(keP[:], GP[:], AF.Exp, scale=2.0 / S)
                ksumP = spool.tile([128, 2], f32, tag="ksumP")
                nc.vector.tensor_reduce(
                    ksumP[:], keP[:].rearrange("p (v j) -> p v j", v=2),
                    axis=mybir.AxisListType.X, op=OP.add)

                treluP = tpool.tile([128, 2048], bf16, tag="treluP")
                tre4 = treluP[:].rearrange("p (q v c) -> p q v c", q=4, v=2)
                for v, o1 in enumerate((out1a, out1b)):
                    nc.scalar.activation(
                        tre4[:, 0:RA // 256, v, :],
                        o1[:, 0:RA].rearrange("p (q c) -> p q c", c=256),
                        AF.Relu)
                    nc.vector.tensor_scalar(
                        tre4[:, RA // 256:4, v, :],
                        o1[:, RA:1024].rearrange("p (q c) -> p q c", c=256),
                        0.0, None, op0=OP.max)

                kspP = spool.tile([128, 64], f32, tag="kspP")
                nc.gpsimd.tensor_copy(kspP[:],
                                      ksumP[:].to_broadcast((128, 2, 32)))
                kTP = spool.tile([128, 64], f32, tag="kTP")
                nc.vector.transpose(kTP[:], kspP[:])
                kmaxP = spool.tile([128, 2], f32, tag="kmaxP")
                nc.vector.tensor_reduce(
                    kmaxP[:], kTP[:].rearrange("p (v y) -> p v y", v=2),
                    axis=mybir.AxisListType.X, op=OP.max)
                krecP = spool.tile([128, 2], f32, tag="krecP")
                nc.vector.reciprocal(krecP[:], kmaxP[:])
                kdnP = s4pool.tile([128, 2], f32, tag="kdnP")
                nc.vector.tensor_tensor(kdnP[:], ksumP[:], krecP[:],
                                        op=OP.mult)
                d["treluP"], d["kdnP"] = treluP, kdnP
                return d

            def stage_b(d, u):
                treluP = d["treluP"]
                out2P = po2.tile([128, 512], f32, tag="out2P")
                for q in range(4):
                    nc.tensor.matmul(out2P[32 * q:32 * q + 32, :], oblk[:],
                                     treluP[:, 512 * q:512 * q + 512],
                                     tile_position=(0, 32 * q))
                o2sP = spool.tile([128, 512], bf16, tag="o2sP")
                nc.scalar.activation(o2sP[:], out2P[:], AF.Copy)
                # bounce out: (q,b) rows -> scr[u, v, b, q, ihd] (all 32 b)
                for v in range(2):
                    nc.sync.dma_start(
                        scr_d[u, v].transpose([1, 0, 2]),
                        o2sP[:, 256 * v:256 * v + 256])

            def stage_c(d, u):
                # bounce in: scr flat -> fexpP[32b+8q+ih, 32v+d]
                fexpP = spool.tile([128, 64], bf16, tag="fexpP")
                for v in range(2):
                    nc.gpsimd.dma_start(
                        fexpP[:, 32 * v:32 * v + 32],
                        scr_d[u, v, 0:4].rearrange(
                            "b q (ih e) -> (b q ih) e", e=32))
                fscP = spool.tile([128, 64], bf16, tag="fscP")
                for v in range(2):
                    nc.vector.tensor_scalar(
                        fscP[:, 32 * v:32 * v + 32],
                        fexpP[:, 32 * v:32 * v + 32],
                        d["kdnP"][:, v:v + 1], None, op0=OP.mult)
                fscTP = s4pool.tile([128, 64], bf16, tag="fscTP")
                nc.vector.transpose(fscTP[:], fscP[:])
                d["fscTP"] = fscTP

            def stage_d(d, u):
                fscTP = d["fscTP"]
                H = phd.tile([128, 64], f32, tag="hdt")
                nc.tensor.matmul(H[:], w1m[:], fscTP[:])
                HS = spool.tile([128, 64], bf16, tag="HS")
                nc.vector.tensor_scalar(HS[:], H[:], b1r[:], 0.0,
                                        op0=OP.add, op1=OP.max)
                DT = phd.tile([128, 64], f32, tag="hdt2")
                nc.tensor.matmul(DT[:], w2m[:], HS[:])
                dls = spool.tile([128, 64], f32, tag="dls")
                nc.vector.tensor_copy(dls[:], DT[:])
                nc.sync.dma_start(out_d[:, 64 * u:64 * u + 64], dls[:])

            pre[0] = stage_pre(0)
            for u in range(nu + 3):
                if u + 1 < nu:
                    pre[u + 1] = stage_pre(u + 1)
                if u < nu:
                    st[u] = stage_a(u)
                if 1 <= u < nu + 1:
                    stage_b(st[u - 1], u - 1)
                if 2 <= u < nu + 2:
                    stage_c(st[u - 2], u - 2)
                if u >= 3:
                    stage_d(st[u - 3], u - 3)
                    del st[u - 3]

    nc.finalize()
    return nc


def _shared_inputs(W_pe, b_pe, W1, b1, W2, b2):
    f32, bf16 = np.float32, ml_dtypes.bfloat16

    # column order: col(i, d) = 256*(i%4) + 32*(i//4) + d
    colbase = np.array([256 * (i % 4) + 32 * (i // 4) for i in range(32)])

    R = np.zeros((100, 1024), f32)
    for i in range(32):
        cb = colbase[i]
        for c in range(3):
            R[3 * i + c, cb:cb + 32] = W_pe[c]
        R[96:99, cb:cb + 32] = -W_pe
        R[99, cb:cb + 32] = b_pe

    oblk = np.zeros((128, 32), f32)
    for b in range(4):
        oblk[32 * b:32 * b + 32, b] = 1.0

    w1m = np.zeros((128, 128), f32)
    b1r = np.zeros((128, 1), f32)
    w2m = np.zeros((128, 128), f32)
    for b in range(4):
        w1m[32 * b:32 * b + 32, 32 * b:32 * b + 16] = W1 / K
        b1r[32 * b:32 * b + 16, 0] = b1
        w2m[32 * b:32 * b + 16, 32 * b:32 * b + 3] = W2

    return {
        "R1": R.astype(bf16), "oblk": oblk.astype(bf16),
        "w1msk": w1m.astype(bf16), "b1r": b1r, "w2msk": w2m.astype(bf16),
    }


def _core_inputs(coords_core, shared, nt=NT):
    f32, bf16 = np.float32, ml_dtypes.bfloat16
    pts = coords_core.reshape(-1, 3).astype(f32)          # [nt*128, 3]
    npts = pts.shape[0]

    # xv[3i+c, 4t+b] = pts[128t+32b+i, c]
    cb = pts.reshape(nt, 4, 32, 3)
    xv = cb.transpose(2, 3, 0, 1).reshape(96, nt * 4)

    ones = np.ones((1, npts), f32)
    cttb = np.concatenate([pts.T, ones], 0)               # [4, npts]

    # gram/kde partition order within each 32-block: pos 8q+ih <- pt 4*ih+q
    pos = np.arange(32)
    src_i = 4 * (pos % 8) + pos // 8
    gidx = (np.arange(npts).reshape(-1, 32)[:, src_i]).reshape(-1)
    ptsg = pts[gidx]
    n2hg = (-0.5 * (ptsg ** 2).sum(-1))[None, :]
    bid = np.broadcast_to(
        (np.arange(npts) // 32 % 4).astype(f32), (1, npts))[:, gidx]
    # contract-8 gram with exact-in-bf16 neighborhood mask:
    # lhsT rows [c, 1, n2h, bid^2, bid, 1]; rhs [c, n2h, 1, -M, 2M*bid, -M*bid^2]
    ctgA = np.concatenate([ptsg.T, ones, n2hg, bid ** 2, bid, ones], 0)
    ctgB = np.concatenate([ptsg.T, n2hg, ones, -MBID * ones,
                           2.0 * MBID * bid, -MBID * bid ** 2], 0)
    # interleave per-supertile: [A-pair(256) | B-pair(256)]
    nu = nt // 2
    ctgAB = np.empty((8, nu, 512), f32)
    ctgAB[:, :, 0:256] = ctgA.reshape(8, nu, 256)
    ctgAB[:, :, 256:512] = ctgB.reshape(8, nu, 256)

    return {
        "xv": xv.astype(bf16), "cttb": cttb.astype(bf16),
        "ctgAB": ctgAB.reshape(8, nt * 256).astype(bf16), **shared,
    }


def _postprocess(delta_raw, b2, nt=NT):
    """Device output [128, nt*32] -> [nt*4, K, 3] (adds b2).

    Device column 8q+ih within a block holds point i = 4*ih+q."""
    o = np.asarray(delta_raw, np.float32).reshape(4, 32, nt, 4, 8)
    out = o[:, 0:3]                                       # [b, e, t, q, ih]
    out = out.transpose(2, 0, 4, 3, 1)                    # [t, b, ih, q, e]
    return (out.reshape(nt * 4, K, 3) + b2[None, None, :]).astype(np.float32)


def _kernel_numpy_small(coords, W_pe, b_pe, W1, b1, W2, b2):
    out = np.empty((coords.shape[0], K, 3), np.float32)
    for c0 in range(0, coords.shape[0], 512):
        c = coords[c0:c0 + 512].astype(np.float32)
        rel = c[:, :, None, :] - c[:, None, :, :]
        pe = np.maximum(rel @ W_pe + b_pe, 0.0).mean(2)
        d2 = (rel * rel).sum(-1)
        kde = COEF * np.exp(-d2 / S).sum(2)[..., None]
        kde = kde / (kde.max(1, keepdims=True) + EPS)
        h = np.maximum((pe * kde) @ W1 + b1, 0.0)
        out[c0:c0 + 512] = h @ W2 + b2
    return out


def _kernel_numpy(coords, W_pe, b_pe, W1, b1, W2, b2):
    return _kernel_numpy_small(coords, W_pe, b_pe, W1, b1, W2, b2)


def kernel(coords, W_pe, b_pe, W1, b1, W2, b2, _trace=False):
    coords = np.asarray(coords, np.float32)
    try:
        from concourse.bass_utils import run_bass_kernel_spmd

        if "nc" not in _CACHE:
            _CACHE["nc"] = _build_program()
        nc = _CACHE["nc"]
        shared = _shared_inputs(np.asarray(W_pe, np.float32),
                                np.asarray(b_pe, np.float32),
                                np.asarray(W1, np.float32),
                                np.asarray(b1, np.float32),
                                np.asarray(W2, np.float32),
                                np.asarray(b2, np.float32))
        in_maps = [
            _core_inputs(coords[c * BNC:(c + 1) * BNC], shared)
            for c in range(NCORES)
        ]
        res = run_bass_kernel_spmd(nc, in_maps, list(range(NCORES)),
                                   trace=_trace)
        b2f = np.asarray(b2, np.float32)
        out = np.concatenate(
            [_postprocess(r["delta"], b2f) for r in res.results], 0)
        if _trace:
            return out, res
        return out
    except Exception:
        if os.environ.get("KERNEL_NO_FALLBACK"):
            raise
        out = _kernel_numpy(coords, W_pe, b_pe, W1, b1, W2, b2)
        if _trace:
            return out, None
        return out


# revision 30
# speedup vs baseline: 1.4906x; 1.0277x over previous
"""Trainium2 Bass kernel for GaussianKDEOffsetGenerator.

Math (per neighborhood b of K=32 points, PE=32, HID=16):
  p = coords @ W_pe
  pe_sum[i] = sum_j relu(p_i - p_j + b_pe)          (1/K folded into W1)
  kde[i]  = sum_j exp(-|c_i-c_j|^2 / S)             (coef cancels in norm)
  kde_n   = kde / max_i kde
  delta   = relu((pe_sum/K * kde_n) @ W1 + b1) @ W2 + b2

Data parallel over BN=8192 -> 1024 nbhd/core; supertiles of 8 nbhd =
2 subtiles x (4 nbhd = 128 points on partitions).  Within a 32-block,
point order is permuted: position y = 8q+ih holds point i = 4*ih+q.

Per subtile (partitions (b,j), free col(i,d) = 256q + 32*ih + d):
  mm1 (bf16, contract 100): rows 0:96 block-bcast coords (xv) paired with
    R rows delta_{i,i'}W_pe; 96:99 coords paired with -W_pe; 99 ones/b_pe.
  gram-full (bf16, contract 8, ONE matmul): G = c_i.c_j - (n_i+n_j)/2
    - M*(bid_i-bid_j)^2; the bid mask zeroes cross-neighborhood exp terms
    so one [128,128] matmul + one paired exp works; mask rows are
    bf16-exact (bid in {0..3}, M=60).
  ksum = DVE segmented reduce of exp;  block max via bcast/transpose/max.
  relu: split ACT (cols 0:RA) / DVE (RA:1024) -> treluP bf16.
  mm2 x4 (3D-AP rhs over the supertile pair) -> out2P[32q+b, ...] psum;
    ACT copy -> o2sP; DMA bounce through DRAM scratch (2 out SP,
    2 back Pool) realizes the (q,b)x(ih) -> (b,q,ih) partition rearrange:
    fexpP[32b+8q+ih, 32v+d] = pe_sum.
  fsc = fexp*ksum*krec; fscT = 32-block transpose -> [(b,d), (v,y)].
  MLP via block-diagonal masked weights, ONE matmul each:
    H = W1msk^T @ fscT; HS = relu(H + b1r); DT = W2msk^T @ HS -> out.
  Host: delta[4t+b, i=4*ih+q, e] = out[32b+e, 32t + 8q+ih] + b2[e].

4-stage pipeline: A(u) loads/gram/mm1/exp/relu/kde; B(u-1) mm2/o2s/dma1;
C(u-2) dma2/fsc/fscT; D(u-3) mm3/HS/mm4/dls/store.
"""

import math
import os
import numpy as np
import ml_dtypes

BN, K, PE_DIM, HID = 8192, 32, 32, 16
SIGMA, EPS = 1.0, 1e-6
NCORES = 8
BNC = BN // NCORES          # 1024 neighborhoods per core
NT = BNC // 4               # 256 subtiles of 4 neighborhoods
S = 2.0 * SIGMA ** 2 + EPS
COEF = 1.0 / math.sqrt(2.0 * math.pi * SIGMA ** 2 + EPS)
RA = 768                    # relu columns per subtile done on ACT
MBID = 60.0                 # cross-neighborhood gram mask

_CACHE = {}


def _build_program(nt=NT):
    import concourse.mybir as mybir
    from concourse import tile
    from concourse.bacc import Bacc

    f32 = mybir.dt.float32
    bf16 = mybir.dt.bfloat16
    AF = mybir.ActivationFunctionType
    OP = mybir.AluOpType
    nu = nt // 2

    nc = Bacc()

    # ---- DRAM I/O ----
    xv_d = nc.declare_dram_parameter("xv", [96, nt * 4], bf16, isOutput=False)
    r1_d = nc.declare_dram_parameter("R1", [100, 1024], bf16, isOutput=False)
    ctb_d = nc.declare_dram_parameter("cttb", [4, nt * 128], bf16, isOutput=False)
    cgw_d = nc.declare_dram_parameter("ctgAB", [8, nt * 256], bf16, isOutput=False)
    ob_d = nc.declare_dram_parameter("oblk", [128, 32], bf16, isOutput=False)
    w1_d = nc.declare_dram_parameter("w1msk", [128, 128], bf16, isOutput=False)
    b1_d = nc.declare_dram_parameter("b1r", [128, 1], f32, isOutput=False)
    w2_d = nc.declare_dram_parameter("w2msk", [128, 128], bf16, isOutput=False)
    out_d = nc.declare_dram_parameter("delta", [128, nt * 32], f32, isOutput=True)
    # bounce scratch: per (supertile u, subtile v): [4(b), 4(q), 256(ih,d)]
    scr_d = nc.dram_tensor("scr", [nu, 2, 32, 4, 256], bf16)

    with tile.TileContext(nc) as tc:
        with (
            tc.tile_pool(name="const", bufs=1) as cpool,
            tc.tile_pool(name="lhs", bufs=3) as lpool,
            tc.tile_pool(name="trelu", bufs=3) as tpool,
            tc.tile_pool(name="sm", bufs=2) as spool,
            tc.tile_pool(name="sm4", bufs=4) as s4pool,
            tc.tile_pool(name="po1", bufs=1, space="PSUM") as po1,
            tc.tile_pool(name="pg", bufs=1, space="PSUM") as pg,
            tc.tile_pool(name="po2", bufs=2, space="PSUM") as po2,
            tc.tile_pool(name="phd", bufs=1, space="PSUM") as phd,
        ):
            # persistent constants (resident all run)
            xv = cpool.tile([96, nt * 4], bf16)
            rhs1 = cpool.tile([100, 1024], bf16)
            cttb = cpool.tile([4, nt * 128], bf16)
            oblk = cpool.tile([128, 32], bf16)
            w1m = cpool.tile([128, 128], bf16)
            b1r = cpool.tile([128, 1], f32)
            w2m = cpool.tile([128, 128], bf16)
            for t_, d_ in [(xv, xv_d), (rhs1, r1_d), (cttb, ctb_d),
                           (oblk, ob_d), (w1m, w1_d), (b1r, b1_d),
                           (w2m, w2_d)]:
                nc.sync.dma_start(t_[:], d_[:])

            st = {}
            pre = {}

            def stage_pre(u):
                c0 = 256 * u                     # point-column base (pair)
                ctgW = lpool.tile([8, 512], bf16, tag="ctgW")
                nc.sync.dma_start(ctgW[:], cgw_d[:, 512 * u:512 * u + 512])
                lhsT1 = lpool.tile([100, 256], bf16, tag="lhsT1")
                nc.vector.tensor_copy(
                    lhsT1[0:96, :].rearrange("p (s b j) -> p s b j", s=2, b=4),
                    xv[:, 8 * u:8 * u + 8].rearrange(
                        "p (s b) -> p s b", s=2).to_broadcast((96, 2, 4, 32)))
                nc.vector.tensor_copy(lhsT1[96:100, :], cttb[:, c0:c0 + 256])
                return ctgW, lhsT1

            def stage_a(u):
                d = {}
                ctgW, lhsT1 = pre.pop(u)

                GP = pg.tile([128, 256], f32, tag="G")
                for v in range(2):
                    nc.tensor.matmul(GP[:, 128 * v:128 * v + 128],
                                     ctgW[:, 128 * v:128 * v + 128],
                                     ctgW[:, 256 + 128 * v:384 + 128 * v])

                out1a = po1.tile([128, 1024], f32, tag="o1a")
                out1b = po1.tile([128, 1024], f32, tag="o1b")
                for v, o1 in enumerate((out1a, out1b)):
                    nc.tensor.matmul(o1[:, 0:512], lhsT1[:, 128 * v:128 * v + 128],
                                     rhs1[:, 0:512])
                    nc.tensor.matmul(o1[:, 512:1024],
                                     lhsT1[:, 128 * v:128 * v + 128],
                                     rhs1[:, 512:1024])

                keP = spool.tile([128, 256], bf16, tag="keP")
                ksumP = spool.tile([128, 2], f32, tag="ksumP")
                for v in range(2):
                    nc.scalar.activation(keP[:, 128 * v:128 * v + 128],
                                         GP[:, 128 * v:128 * v + 128],
                                         AF.Exp, scale=2.0 / S,
                                         accum_out=ksumP[:, v:v + 1])

                treluP = tpool.tile([128, 2048], bf16, tag="treluP")
                tre4 = treluP[:].rearrange("p (q v c) -> p q v c", q=4, v=2)
                for v, o1 in enumerate((out1a, out1b)):
                    nc.scalar.activation(
                        tre4[:, 0:RA // 256, v, :],
                        o1[:, 0:RA].rearrange("p (q c) -> p q c", c=256),
                        AF.Relu)
                    nc.vector.tensor_scalar(
                        tre4[:, RA // 256:4, v, :],
                        o1[:, RA:1024].rearrange("p (q c) -> p q c", c=256),
                        0.0, None, op0=OP.max)

                kspP = spool.tile([128, 64], f32, tag="kspP")
                nc.gpsimd.tensor_copy(kspP[:],
                                      ksumP[:].to_broadcast((128, 2, 32)))
                kTP = spool.tile([128, 64], f32, tag="kTP")
                nc.vector.transpose(kTP[:], kspP[:])
                kmaxP = spool.tile([128, 2], f32, tag="kmaxP")
                nc.vector.tensor_reduce(
                    kmaxP[:], kTP[:].rearrange("p (v y) -> p v y", v=2),
                    axis=mybir.AxisListType.X, op=OP.max)
                krecP = spool.tile([128, 2], f32, tag="krecP")
                nc.vector.reciprocal(krecP[:], kmaxP[:])
                kdnP = s4pool.tile([128, 2], f32, tag="kdnP")
                nc.vector.tensor_tensor(kdnP[:], ksumP[:], krecP[:],
                                        op=OP.mult)
                d["treluP"], d["kdnP"] = treluP, kdnP
                return d

            def stage_b(d, u):
                treluP = d["treluP"]
                out2P = po2.tile([128, 512], f32, tag="out2P")
                for q in range(4):
                    nc.tensor.matmul(out2P[32 * q:32 * q + 32, :], oblk[:],
                                     treluP[:, 512 * q:512 * q + 512],
                                     tile_position=(0, 32 * q))
                o2sP = spool.tile([128, 512], bf16, tag="o2sP")
                nc.scalar.activation(o2sP[:], out2P[:], AF.Copy)
                # bounce out: (q,b) rows -> scr[u, v, b, q, ihd] (all 32 b)
                for v in range(2):
                    nc.sync.dma_start(
                        scr_d[u, v].transpose([1, 0, 2]),
                        o2sP[:, 256 * v:256 * v + 256])

            def stage_c(d, u):
                # bounce in: scr flat -> fexpP[32b+8q+ih, 32v+d]
                fexpP = spool.tile([128, 64], bf16, tag="fexpP")
                for v in range(2):
                    nc.gpsimd.dma_start(
                        fexpP[:, 32 * v:32 * v + 32],
                        scr_d[u, v, 0:4].rearrange(
                            "b q (ih e) -> (b q ih) e", e=32))
                fscP = spool.tile([128, 64], bf16, tag="fscP")
                for v in range(2):
                    nc.vector.tensor_scalar(
                        fscP[:, 32 * v:32 * v + 32],
                        fexpP[:, 32 * v:32 * v + 32],
                        d["kdnP"][:, v:v + 1], None, op0=OP.mult)
                fscTP = s4pool.tile([128, 64], bf16, tag="fscTP")
                nc.vector.transpose(fscTP[:], fscP[:])
                d["fscTP"] = fscTP

            def stage_d(d, u):
                fscTP = d["fscTP"]
                H = phd.tile([128, 64], f32, tag="hdt")
                nc.tensor.matmul(H[:], w1m[:], fscTP[:])
                HS = spool.tile([128, 64], bf16, tag="HS")
                nc.vector.tensor_scalar(HS[:], H[:], b1r[:], 0.0,
                                        op0=OP.add, op1=OP.max)
                DT = phd.tile([128, 64], f32, tag="hdt")
                nc.tensor.matmul(DT[:], w2m[:], HS[:])
                dls = spool.tile([128, 64], f32, tag="dls")
                nc.vector.tensor_copy(dls[:], DT[:])
                nc.sync.dma_start(out_d[:, 64 * u:64 * u + 64], dls[:])

            pre[0] = stage_pre(0)
            for u in range(nu + 4):
                if u + 1 < nu:
                    pre[u + 1] = stage_pre(u + 1)
                if u < nu:
                    st[u] = stage_a(u)
                if 2 <= u < nu + 2:
                    stage_b(st[u - 2], u - 2)
                if 3 <= u < nu + 3:
                    stage_c(st[u - 3], u - 3)
                if u >= 4:
                    stage_d(st[u - 4], u - 4)
                    del st[u - 4]

    nc.finalize()
    return nc


def _shared_inputs(W_pe, b_pe, W1, b1, W2, b2):
    f32, bf16 = np.float32, ml_dtypes.bfloat16

    # column order: col(i, d) = 256*(i%4) + 32*(i//4) + d
    colbase = np.array([256 * (i % 4) + 32 * (i // 4) for i in range(32)])

    R = np.zeros((100, 1024), f32)
    for i in range(32):
        cb = colbase[i]
        for c in range(3):
            R[3 * i + c, cb:cb + 32] = W_pe[c]
        R[96:99, cb:cb + 32] = -W_pe
        R[99, cb:cb + 32] = b_pe

    oblk = np.zeros((128, 32), f32)
    for b in range(4):
        oblk[32 * b:32 * b + 32, b] = 1.0

    w1m = np.zeros((128, 128), f32)
    b1r = np.zeros((128, 1), f32)
    w2m = np.zeros((128, 128), f32)
    for b in range(4):
        w1m[32 * b:32 * b + 32, 32 * b:32 * b + 16] = W1 / K
        b1r[32 * b:32 * b + 16, 0] = b1
        w2m[32 * b:32 * b + 16, 32 * b:32 * b + 3] = W2

    return {
        "R1": R.astype(bf16), "oblk": oblk.astype(bf16),
        "w1msk": w1m.astype(bf16), "b1r": b1r, "w2msk": w2m.astype(bf16),
    }


def _core_inputs(coords_core, shared, nt=NT):
    f32, bf16 = np.float32, ml_dtypes.bfloat16
    pts = coords_core.reshape(-1, 3).astype(f32)          # [nt*128, 3]
    npts = pts.shape[0]

    # xv[3i+c, 4t+b] = pts[128t+32b+i, c]
    cb = pts.reshape(nt, 4, 32, 3)
    xv = cb.transpose(2, 3, 0, 1).reshape(96, nt * 4)

    ones = np.ones((1, npts), f32)
    cttb = np.concatenate([pts.T, ones], 0)               # [4, npts]

    # gram/kde partition order within each 32-block: pos 8q+ih <- pt 4*ih+q
    pos = np.arange(32)
    src_i = 4 * (pos % 8) + pos // 8
    gidx = (np.arange(npts).reshape(-1, 32)[:, src_i]).reshape(-1)
    ptsg = pts[gidx]
    n2hg = (-0.5 * (ptsg ** 2).sum(-1))[None, :]
    bid = np.broadcast_to(
        (np.arange(npts) // 32 % 4).astype(f32), (1, npts))[:, gidx]
    # contract-8 gram with exact-in-bf16 neighborhood mask:
    # lhsT rows [c, 1, n2h, bid^2, bid, 1]; rhs [c, n2h, 1, -M, 2M*bid, -M*bid^2]
    ctgA = np.concatenate([ptsg.T, ones, n2hg, bid ** 2, bid, ones], 0)
    ctgB = np.concatenate([ptsg.T, n2hg, ones, -MBID * ones,
                           2.0 * MBID * bid, -MBID * bid ** 2], 0)
    # interleave per-supertile: [A-pair(256) | B-pair(256)]
    nu = nt // 2
    ctgAB = np.empty((8, nu, 512), f32)
    ctgAB[:, :, 0:256] = ctgA.reshape(8, nu, 256)
    ctgAB[:, :, 256:512] = ctgB.reshape(8, nu, 256)

    return {
        "xv": xv.astype(bf16), "cttb": cttb.astype(bf16),
        "ctgAB": ctgAB.reshape(8, nt * 256).astype(bf16), **shared,
    }


def _postprocess(delta_raw, b2, nt=NT):
    """Device output [128, nt*32] -> [nt*4, K, 3] (adds b2).

    Device column 8q+ih within a block holds point i = 4*ih+q."""
    o = np.asarray(delta_raw, np.float32).reshape(4, 32, nt, 4, 8)
    out = o[:, 0:3]                                       # [b, e, t, q, ih]
    out = out.transpose(2, 0, 4, 3, 1)                    # [t, b, ih, q, e]
    return (out.reshape(nt * 4, K, 3) + b2[None, None, :]).astype(np.float32)


def _kernel_numpy_small(coords, W_pe, b_pe, W1, b1, W2, b2):
    out = np.empty((coords.shape[0], K, 3), np.float32)
    for c0 in range(0, coords.shape[0], 512):
        c = coords[c0:c0 + 512].astype(np.float32)
        rel = c[:, :, None, :] - c[:, None, :, :]
        pe = np.maximum(rel @ W_pe + b_pe, 0.0).mean(2)
        d2 = (rel * rel).sum(-1)
        kde = COEF * np.exp(-d2 / S).sum(2)[..., None]
        kde = kde / (kde.max(1, keepdims=True) + EPS)
        h = np.maximum((pe * kde) @ W1 + b1, 0.0)
        out[c0:c0 + 512] = h @ W2 + b2
    return out


def _kernel_numpy(coords, W_pe, b_pe, W1, b1, W2, b2):
    return _kernel_numpy_small(coords, W_pe, b_pe, W1, b1, W2, b2)


def kernel(coords, W_pe, b_pe, W1, b1, W2, b2, _trace=False):
    coords = np.asarray(coords, np.float32)
    try:
        from concourse.bass_utils import run_bass_kernel_spmd

        if "nc" not in _CACHE:
            _CACHE["nc"] = _build_program()
        nc = _CACHE["nc"]
        shared = _shared_inputs(np.asarray(W_pe, np.float32),
                                np.asarray(b_pe, np.float32),
                                np.asarray(W1, np.float32),
                                np.asarray(b1, np.float32),
                                np.asarray(W2, np.float32),
                                np.asarray(b2, np.float32))
        in_maps = [
            _core_inputs(coords[c * BNC:(c + 1) * BNC], shared)
            for c in range(NCORES)
        ]
        res = run_bass_kernel_spmd(nc, in_maps, list(range(NCORES)),
                                   trace=_trace)
        b2f = np.asarray(b2, np.float32)
        out = np.concatenate(
            [_postprocess(r["delta"], b2f) for r in res.results], 0)
        if _trace:
            return out, res
        return out
    except Exception:
        if os.environ.get("KERNEL_NO_FALLBACK"):
            raise
        out = _kernel_numpy(coords, W_pe, b_pe, W1, b1, W2, b2)
        if _trace:
            return out, None
        return out
